# revision 5
# baseline (speedup 1.0000x reference)
"""Trainium2 Bass kernel for nn_HeatEquation1D.

The reference applies a fixed 62x62 Crank-Nicolson step matrix 100 times to
u0[:, 1:-1] via lax.scan, then zero-pads the boundary columns.  Algebraically
that whole scan is a single matmul:

    out = u0 @ W64,   W64[1:63, 1:63] = (step_matrix^100).T,  zero elsewhere

W64 is computed on the host in float64.  The rel-err budget (2e-2) admits
bf16: the host rounds u0 to bf16 (halving the device's read traffic), the
matmul runs bf16 x bf16 -> f32, and the device stores bf16 which the host
upcasts to f32 (halving write traffic).  Measured rel err ~2.8e-3.

Device kernel (per core, pure data parallel over 8 cores):
  - u (bf16, 65536 x 64) is viewed as [32768 row-pairs, 128] and loaded with
    HW DMA-transpose (sync ring) into SBUF tiles T = [128, 1024]: partition
    k = (r, f) interleaved (r = row-within-pair, f = feature), free = row-pair.
    This IS the transposed-chunk layout the matmul needs -- no PE transposes,
    no PSUM round-trip for the transpose.
  - One constant stationary BD = block_diag(W64, W64) (bf16, symmetric);
    matmul(yp, lhsT=BD, rhs=T[:, j:j+512]) gives Y in the same interleaved
    layout: yp[(r, f'), n] = (u @ W64)[2n+r, f'].
  - DVE and ACT alternately copy yp (PSUM f32) -> ys (SBUF bf16).
  - Stores (scalar ring, overlapping the loads' sync ring) write ys to a
    bf16 DRAM tensor out[128, 32768]; the host un-interleaves
    out[(r, f'), p] -> y[2p+r, f'] and upcasts to f32.

Per-core HBM traffic: 2 x 8.39 MB; the two DMA streams overlap on separate
HWDGE rings.  32 blocks of 256 KiB with deep tile pools keep the pipeline
throughput-bound instead of latency-bound.
"""

import numpy as np
import ml_dtypes

BATCH = 524288
NX = 64
NUM_STEPS = 100
N_CORES = 8
ROWS_PER_CORE = BATCH // N_CORES          # 65536
P = 128
PAIRS = ROWS_PER_CORE // 2                 # 32768 row-pairs per core

BLOCK_PAIRS = 1024                         # row-pairs per DMA block (256 KiB)
N_BLOCKS = PAIRS // BLOCK_PAIRS            # 32
MM_N = 512                                 # matmul moving free dim (fp32 out -> 1 bank)

TRACE = False
LAST_RESULTS = None

_NC_CACHE = {}


def _build_nc():
    from concourse import bacc, mybir
    from concourse.tile import TileContext

    nc = bacc.Bacc("TRN2", target_bir_lowering=False, debug=False)
    f32 = mybir.dt.float32
    bf16 = mybir.dt.bfloat16

    u = nc.dram_tensor("u", [ROWS_PER_CORE, NX], bf16, kind="ExternalInput")
    bd_d = nc.dram_tensor("bd", [P, P], bf16, kind="ExternalInput")
    out = nc.dram_tensor("out", [P, PAIRS], bf16, kind="ExternalOutput")

    u_r = u.rearrange("(nb rp r) f -> nb rp (r f)", rp=BLOCK_PAIRS, r=2)
    out_r = out.rearrange("k (nb rp) -> nb k rp", rp=BLOCK_PAIRS)

    with TileContext(nc) as tc:
        with (
            tc.tile_pool(name="consts", bufs=1) as cpool,
            tc.tile_pool(name="xin", bufs=6) as xpool,
            tc.tile_pool(name="yout", bufs=4) as ypool,
            tc.tile_pool(name="ps_y", bufs=4, space="PSUM") as psy,
        ):
            bd_s = cpool.tile([P, P], bf16)
            nc.scalar.dma_start(out=bd_s[:], in_=bd_d[:])

            for nb in range(N_BLOCKS):
                x = xpool.tile([P, BLOCK_PAIRS], bf16)
                nc.sync.dma_start(out=x[:], in_=u_r[nb], transpose=True)

                ys = ypool.tile([P, BLOCK_PAIRS], bf16)
                yp = psy.tile([P, 2, MM_N], f32)
                for m in range(2):
                    nc.tensor.matmul(
                        yp[:, m],
                        bd_s[:],
                        x[:, m * MM_N : (m + 1) * MM_N],
                        start=True,
                        stop=True,
                    )
                if nb % 2 == 0:
                    nc.vector.tensor_copy(out=ys[:], in_=yp[:])
                else:
                    nc.scalar.copy(out=ys[:], in_=yp[:])
                nc.scalar.dma_start(out=out_r[nb], in_=ys[:])

    nc.compile()
    return nc


def _host_matrix(step_matrix):
    m = np.asarray(step_matrix, dtype=np.float64)
    w_inner = np.linalg.matrix_power(m, NUM_STEPS).T  # right-multiplier, f64
    w64 = np.zeros((NX, NX), dtype=np.float64)
    w64[1 : NX - 1, 1 : NX - 1] = w_inner
    bd = np.zeros((P, P), dtype=np.float64)
    bd[:NX, :NX] = w64
    bd[NX:, NX:] = w64
    return bd.astype(ml_dtypes.bfloat16)


def kernel(u0, step_matrix):
    global LAST_RESULTS
    from concourse.bass_utils import run_bass_kernel_spmd

    u0 = np.asarray(u0)
    assert u0.shape == (BATCH, NX), u0.shape
    u0_bf = np.ascontiguousarray(u0.astype(ml_dtypes.bfloat16))

    bd = _host_matrix(step_matrix)

    if "nc" not in _NC_CACHE:
        _NC_CACHE["nc"] = _build_nc()
    nc = _NC_CACHE["nc"]

    shards = np.split(u0_bf, N_CORES, axis=0)
    in_maps = [{"u": s, "bd": bd} for s in shards]
    res = run_bass_kernel_spmd(
        nc, in_maps, core_ids=list(range(N_CORES)), trace=TRACE
    )
    LAST_RESULTS = res

    outs = []
    for r in res.results:
        arr = np.asarray(r["out"])  # [128, 32768] bf16, k=(r,f) interleaved
        y = (
            arr.reshape(2, NX, PAIRS)
            .transpose(2, 0, 1)
            .reshape(ROWS_PER_CORE, NX)
            .astype(np.float32)
        )
        outs.append(y)
    return np.concatenate(outs, axis=0)


# revision 7
# speedup vs baseline: 1.5109x; 1.5109x over previous
"""Trainium2 Bass kernel for nn_HeatEquation1D.

The reference applies a fixed 62x62 Crank-Nicolson step matrix 100 times to
u0[:, 1:-1] via lax.scan, then zero-pads the boundary columns.  Algebraically
that whole scan is a single matmul:

    out = u0 @ W64,   W64[1:63, 1:63] = (step_matrix^100).T,  zero elsewhere

W64 is computed on the host in float64.  The rel-err budget (2e-2) admits
bf16: the host rounds u0 to bf16 (halving the device's read traffic), the
matmul runs bf16 x bf16 -> f32, and the device stores bf16 which the host
upcasts to f32 (halving write traffic).  Measured rel err ~2.8e-3.

Device kernel (per core, pure data parallel over 8 cores):
  - u (bf16, 65536 x 64) is viewed as [32768 row-pairs, 128] and loaded with
    HW DMA-transpose (sync ring) into SBUF tiles T = [128, 1024]: partition
    k = (r, f) interleaved (r = row-within-pair, f = feature), free = row-pair.
    This IS the transposed-chunk layout the matmul needs -- no PE transposes,
    no PSUM round-trip for the transpose.
  - One constant stationary BD = block_diag(W64, W64) (bf16, symmetric);
    matmul(yp, lhsT=BD, rhs=T[:, j:j+512]) gives Y in the same interleaved
    layout: yp[(r, f'), n] = (u @ W64)[2n+r, f'].
  - DVE and ACT alternately copy yp (PSUM f32) -> ys (SBUF bf16).
  - Stores (scalar ring, overlapping the loads' sync ring) write ys to a
    bf16 DRAM tensor out[128, 32768]; the host un-interleaves
    out[(r, f'), p] -> y[2p+r, f'] and upcasts to f32.

Per-core HBM traffic: 2 x 8.39 MB; the two DMA streams overlap on separate
HWDGE rings.  32 blocks of 256 KiB with deep tile pools keep the pipeline
throughput-bound instead of latency-bound.
"""

import numpy as np
import ml_dtypes

BATCH = 524288
NX = 64
NUM_STEPS = 100
N_CORES = 8
ROWS_PER_CORE = BATCH // N_CORES          # 65536
P = 128
PAIRS = ROWS_PER_CORE // 2                 # 32768 row-pairs per core

BLOCK_PAIRS = 2048                         # row-pairs per DMA block (512 KiB)
N_BLOCKS = PAIRS // BLOCK_PAIRS            # 16
MM_N = 512                                 # matmul moving free dim (fp32 out -> 1 bank)

TRACE = False
LAST_RESULTS = None

_NC_CACHE = {}


def _build_nc():
    from concourse import bacc, mybir
    from concourse.tile import TileContext

    nc = bacc.Bacc("TRN2", target_bir_lowering=False, debug=False)
    f32 = mybir.dt.float32
    bf16 = mybir.dt.bfloat16

    u = nc.dram_tensor("u", [ROWS_PER_CORE, NX], bf16, kind="ExternalInput")
    bd_d = nc.dram_tensor("bd", [P, P], bf16, kind="ExternalInput")
    out = nc.dram_tensor("out", [P, PAIRS], bf16, kind="ExternalOutput")

    u_r = u.rearrange("(nb rp r) f -> nb rp (r f)", rp=BLOCK_PAIRS, r=2)
    out_r = out.rearrange("k (nb rp) -> nb k rp", rp=BLOCK_PAIRS)

    with TileContext(nc) as tc:
        with (
            tc.tile_pool(name="consts", bufs=1) as cpool,
            tc.tile_pool(name="xin", bufs=8) as xpool,
            tc.tile_pool(name="yout", bufs=6) as ypool,
            tc.tile_pool(name="ps_y", bufs=8, space="PSUM") as psy,
        ):
            bd_s = cpool.tile([P, P], bf16)
            nc.scalar.dma_start(out=bd_s[:], in_=bd_d[:])

            for nb in range(N_BLOCKS):
                x = xpool.tile([P, BLOCK_PAIRS], bf16)
                nc.sync.dma_start(out=x[:], in_=u_r[nb], transpose=True)

                ys = ypool.tile([P, BLOCK_PAIRS], bf16)
                for m in range(BLOCK_PAIRS // MM_N):
                    yp = psy.tile([P, MM_N], f32)
                    nc.tensor.matmul(
                        yp[:],
                        bd_s[:],
                        x[:, m * MM_N : (m + 1) * MM_N],
                        start=True,
                        stop=True,
                    )
                    dst = ys[:, m * MM_N : (m + 1) * MM_N]
                    if m % 2 == 0:
                        nc.vector.tensor_copy(out=dst, in_=yp[:])
                    else:
                        nc.scalar.copy(out=dst, in_=yp[:])
                nc.scalar.dma_start(out=out_r[nb], in_=ys[:])

    nc.compile()
    return nc


def _host_matrix(step_matrix):
    m = np.asarray(step_matrix, dtype=np.float64)
    w_inner = np.linalg.matrix_power(m, NUM_STEPS).T  # right-multiplier, f64
    w64 = np.zeros((NX, NX), dtype=np.float64)
    w64[1 : NX - 1, 1 : NX - 1] = w_inner
    bd = np.zeros((P, P), dtype=np.float64)
    bd[:NX, :NX] = w64
    bd[NX:, NX:] = w64
    return bd.astype(ml_dtypes.bfloat16)


def kernel(u0, step_matrix):
    global LAST_RESULTS
    from concourse.bass_utils import run_bass_kernel_spmd

    u0 = np.asarray(u0)
    assert u0.shape == (BATCH, NX), u0.shape
    u0_bf = np.ascontiguousarray(u0.astype(ml_dtypes.bfloat16))

    bd = _host_matrix(step_matrix)

    if "nc" not in _NC_CACHE:
        _NC_CACHE["nc"] = _build_nc()
    nc = _NC_CACHE["nc"]

    shards = np.split(u0_bf, N_CORES, axis=0)
    in_maps = [{"u": s, "bd": bd} for s in shards]
    res = run_bass_kernel_spmd(
        nc, in_maps, core_ids=list(range(N_CORES)), trace=TRACE
    )
    LAST_RESULTS = res

    outs = []
    for r in res.results:
        arr = np.asarray(r["out"])  # [128, 32768] bf16, k=(r,f) interleaved
        y = (
            arr.reshape(2, NX, PAIRS)
            .transpose(2, 0, 1)
            .reshape(ROWS_PER_CORE, NX)
            .astype(np.float32)
        )
        outs.append(y)
    return np.concatenate(outs, axis=0)


# revision 9
# speedup vs baseline: 1.6760x; 1.1093x over previous
"""Trainium2 Bass kernel for nn_HeatEquation1D.

The reference applies a fixed 62x62 Crank-Nicolson step matrix 100 times to
u0[:, 1:-1] via lax.scan, then zero-pads the boundary columns.  Algebraically
that whole scan is a single matmul:

    out = u0 @ W64,   W64[1:63, 1:63] = (step_matrix^100).T,  zero elsewhere

W64 is computed on the host in float64.  The rel-err budget (2e-2) admits
bf16: the host rounds u0 to bf16 (halving the device's read traffic), the
matmul runs bf16 x bf16 -> f32, and the device stores bf16 which the host
upcasts to f32 (halving write traffic).  Measured rel err ~2.8e-3.

Device kernel (per core, pure data parallel over 8 cores):
  - u (bf16, 65536 x 64) is viewed as [32768 row-pairs, 128] and loaded with
    HW DMA-transpose (sync ring) into SBUF tiles T = [128, 1024]: partition
    k = (r, f) interleaved (r = row-within-pair, f = feature), free = row-pair.
    This IS the transposed-chunk layout the matmul needs -- no PE transposes,
    no PSUM round-trip for the transpose.
  - One constant stationary BD = block_diag(W64, W64) (bf16, symmetric);
    matmul(yp, lhsT=BD, rhs=T[:, j:j+512]) gives Y in the same interleaved
    layout: yp[(r, f'), n] = (u @ W64)[2n+r, f'].
  - DVE and ACT alternately copy yp (PSUM f32) -> ys (SBUF bf16).
  - Stores (scalar ring, overlapping the loads' sync ring) write ys to a
    bf16 DRAM tensor out[128, 32768]; the host un-interleaves
    out[(r, f'), p] -> y[2p+r, f'] and upcasts to f32.

Per-core HBM traffic: 2 x 8.39 MB; the two DMA streams overlap on separate
HWDGE rings.  32 blocks of 256 KiB with deep tile pools keep the pipeline
throughput-bound instead of latency-bound.
"""

import numpy as np
import ml_dtypes

BATCH = 524288
NX = 64
NUM_STEPS = 100
N_CORES = 8
ROWS_PER_CORE = BATCH // N_CORES          # 65536
P = 128
PAIRS = ROWS_PER_CORE // 2                 # 32768 row-pairs per core

BLOCK_PAIRS = 4096                         # row-pairs per load DMA (1 MiB bf16)
N_BLOCKS = PAIRS // BLOCK_PAIRS            # 8 loads -> exactly 8 HWDGE sem lanes
ST_PAIRS = 2048                            # row-pairs per store DMA (512 KiB)
MM_N = 512                                 # matmul moving free dim (fp32 out -> 1 bank)

TRACE = False
LAST_RESULTS = None

_NC_CACHE = {}


def _build_nc():
    from concourse import bacc, mybir
    from concourse.tile import TileContext

    nc = bacc.Bacc("TRN2", target_bir_lowering=False, debug=False)
    f32 = mybir.dt.float32
    bf16 = mybir.dt.bfloat16

    u = nc.dram_tensor("u", [ROWS_PER_CORE, NX], bf16, kind="ExternalInput")
    bd_d = nc.dram_tensor("bd", [P, P], bf16, kind="ExternalInput")
    out = nc.dram_tensor("out", [P, PAIRS], bf16, kind="ExternalOutput")

    u_r = u.rearrange("(nb rp r) f -> nb rp (r f)", rp=BLOCK_PAIRS, r=2)
    out_r = out.rearrange("k (ns rp) -> ns k rp", rp=ST_PAIRS)

    with TileContext(nc) as tc:
        with (
            tc.tile_pool(name="consts", bufs=1) as cpool,
            tc.tile_pool(name="xin", bufs=3) as xpool,
            tc.tile_pool(name="yout", bufs=4) as ypool,
            tc.tile_pool(name="ps_y", bufs=8, space="PSUM") as psy,
        ):
            bd_s = cpool.tile([P, P], bf16)
            nc.gpsimd.dma_start(out=bd_s[:], in_=bd_d[:])

            for nb in range(N_BLOCKS):
                x = xpool.tile([P, BLOCK_PAIRS], bf16)
                nc.sync.dma_start(out=x[:], in_=u_r[nb], transpose=True)

                for half in range(BLOCK_PAIRS // ST_PAIRS):
                    ys = ypool.tile([P, ST_PAIRS], bf16)
                    for m in range(ST_PAIRS // MM_N):
                        yp = psy.tile([P, MM_N], f32)
                        nc.tensor.matmul(
                            yp[:],
                            bd_s[:],
                            x[:, half * ST_PAIRS + m * MM_N :
                                 half * ST_PAIRS + (m + 1) * MM_N],
                            start=True,
                            stop=True,
                        )
                        dst = ys[:, m * MM_N : (m + 1) * MM_N]
                        if m % 2 == 0:
                            nc.vector.tensor_copy(out=dst, in_=yp[:])
                        else:
                            nc.scalar.copy(out=dst, in_=yp[:])
                    nc.gpsimd.dma_start(
                        out=out_r[nb * (BLOCK_PAIRS // ST_PAIRS) + half],
                        in_=ys[:],
                    )

    nc.compile()
    return nc


def _host_matrix(step_matrix):
    m = np.asarray(step_matrix, dtype=np.float64)
    w_inner = np.linalg.matrix_power(m, NUM_STEPS).T  # right-multiplier, f64
    w64 = np.zeros((NX, NX), dtype=np.float64)
    w64[1 : NX - 1, 1 : NX - 1] = w_inner
    bd = np.zeros((P, P), dtype=np.float64)
    bd[:NX, :NX] = w64
    bd[NX:, NX:] = w64
    return bd.astype(ml_dtypes.bfloat16)


def kernel(u0, step_matrix):
    global LAST_RESULTS
    from concourse.bass_utils import run_bass_kernel_spmd

    u0 = np.asarray(u0)
    assert u0.shape == (BATCH, NX), u0.shape
    u0_bf = np.ascontiguousarray(u0.astype(ml_dtypes.bfloat16))

    bd = _host_matrix(step_matrix)

    if "nc" not in _NC_CACHE:
        _NC_CACHE["nc"] = _build_nc()
    nc = _NC_CACHE["nc"]

    shards = np.split(u0_bf, N_CORES, axis=0)
    in_maps = [{"u": s, "bd": bd} for s in shards]
    res = run_bass_kernel_spmd(
        nc, in_maps, core_ids=list(range(N_CORES)), trace=TRACE
    )
    LAST_RESULTS = res

    outs = []
    for r in res.results:
        arr = np.asarray(r["out"])  # [128, 32768] bf16, k=(r,f) interleaved
        y = (
            arr.reshape(2, NX, PAIRS)
            .transpose(2, 0, 1)
            .reshape(ROWS_PER_CORE, NX)
            .astype(np.float32)
        )
        outs.append(y)
    return np.concatenate(outs, axis=0)


# revision 11
# speedup vs baseline: 1.8312x; 1.0926x over previous
"""Trainium2 Bass kernel for nn_HeatEquation1D.

The reference applies a fixed 62x62 Crank-Nicolson step matrix 100 times to
u0[:, 1:-1] via lax.scan, then zero-pads the boundary columns.  Algebraically
that whole scan is a single matmul:

    out = u0 @ W64,   W64[1:63, 1:63] = (step_matrix^100).T,  zero elsewhere

W64 is computed on the host in float64.  The rel-err budget (2e-2) admits
bf16: the host rounds u0 to bf16 (halving the device's read traffic), the
matmul runs bf16 x bf16 -> f32, and the device stores bf16 which the host
upcasts to f32 (halving write traffic).  Measured rel err ~2.8e-3.

Device kernel (per core, pure data parallel over 8 cores):
  - u (bf16, 65536 x 64) is viewed as [32768 row-pairs, 128] and loaded with
    HW DMA-transpose (sync ring) into SBUF tiles T = [128, 1024]: partition
    k = (r, f) interleaved (r = row-within-pair, f = feature), free = row-pair.
    This IS the transposed-chunk layout the matmul needs -- no PE transposes,
    no PSUM round-trip for the transpose.
  - One constant stationary BD = block_diag(W64, W64) (bf16, symmetric);
    matmul(yp, lhsT=BD, rhs=T[:, j:j+512]) gives Y in the same interleaved
    layout: yp[(r, f'), n] = (u @ W64)[2n+r, f'].
  - DVE and ACT alternately copy yp (PSUM f32) -> ys (SBUF bf16).
  - Stores (scalar ring, overlapping the loads' sync ring) write ys to a
    bf16 DRAM tensor out[128, 32768]; the host un-interleaves
    out[(r, f'), p] -> y[2p+r, f'] and upcasts to f32.

Per-core HBM traffic: 2 x 8.39 MB; the two DMA streams overlap on separate
HWDGE rings.  32 blocks of 256 KiB with deep tile pools keep the pipeline
throughput-bound instead of latency-bound.
"""

import numpy as np
import ml_dtypes

BATCH = 524288
NX = 64
NUM_STEPS = 100
N_CORES = 8
ROWS_PER_CORE = BATCH // N_CORES          # 65536
P = 128
PAIRS = ROWS_PER_CORE // 2                 # 32768 row-pairs per core

BLOCK_PAIRS = 4096                         # row-pairs per load/store DMA (1 MiB bf16)
N_BLOCKS = PAIRS // BLOCK_PAIRS            # 8 loads + 8 stores + 1 const = 17 DMAs
MM_N = 512                                 # matmul moving free dim (fp32 out -> 1 bank)

TRACE = False
LAST_RESULTS = None

_NC_CACHE = {}


def _build_nc():
    from concourse import bacc, mybir
    from concourse.tile import TileContext

    nc = bacc.Bacc("TRN2", target_bir_lowering=False, debug=False)
    f32 = mybir.dt.float32
    bf16 = mybir.dt.bfloat16

    u = nc.dram_tensor("u", [ROWS_PER_CORE, NX], bf16, kind="ExternalInput")
    bd_d = nc.dram_tensor("bd", [P, P], bf16, kind="ExternalInput")
    out = nc.dram_tensor("out", [P, PAIRS], bf16, kind="ExternalOutput")

    u_r = u.rearrange("(nb rp r) f -> nb rp (r f)", rp=BLOCK_PAIRS, r=2)
    out_r = out.rearrange("k (nb rp) -> nb k rp", rp=BLOCK_PAIRS)

    with TileContext(nc) as tc:
        with (
            tc.tile_pool(name="consts", bufs=1) as cpool,
            tc.tile_pool(name="xin", bufs=3) as xpool,
            tc.tile_pool(name="yout", bufs=3) as ypool,
            tc.tile_pool(name="ps_y", bufs=4, space="PSUM") as psy,
        ):
            bd_s = cpool.tile([P, P], bf16)
            nc.gpsimd.dma_start(out=bd_s[:], in_=bd_d[:])

            for nb in range(N_BLOCKS):
                x = xpool.tile([P, BLOCK_PAIRS], bf16)
                ldeng = nc.sync if nb % 2 == 0 else nc.scalar
                ldeng.dma_start(out=x[:], in_=u_r[nb], transpose=True)

                ys = ypool.tile([P, BLOCK_PAIRS], bf16)
                for j in range(BLOCK_PAIRS // (2 * MM_N)):
                    yp = psy.tile([P, 2, MM_N], f32)
                    for m in range(2):
                        c0 = (2 * j + m) * MM_N
                        nc.tensor.matmul(
                            yp[:, m],
                            bd_s[:],
                            x[:, c0 : c0 + MM_N],
                            start=True,
                            stop=True,
                        )
                    dst = ys[:, 2 * j * MM_N : 2 * (j + 1) * MM_N]
                    if j % 2 == 0:
                        nc.vector.tensor_copy(out=dst, in_=yp[:])
                    else:
                        nc.scalar.copy(out=dst, in_=yp[:])
                nc.gpsimd.dma_start(out=out_r[nb], in_=ys[:])

    nc.compile()
    return nc


def _host_matrix(step_matrix):
    m = np.asarray(step_matrix, dtype=np.float64)
    w_inner = np.linalg.matrix_power(m, NUM_STEPS).T  # right-multiplier, f64
    w64 = np.zeros((NX, NX), dtype=np.float64)
    w64[1 : NX - 1, 1 : NX - 1] = w_inner
    bd = np.zeros((P, P), dtype=np.float64)
    bd[:NX, :NX] = w64
    bd[NX:, NX:] = w64
    return bd.astype(ml_dtypes.bfloat16)


def kernel(u0, step_matrix):
    global LAST_RESULTS
    from concourse.bass_utils import run_bass_kernel_spmd

    u0 = np.asarray(u0)
    assert u0.shape == (BATCH, NX), u0.shape
    u0_bf = np.ascontiguousarray(u0.astype(ml_dtypes.bfloat16))

    bd = _host_matrix(step_matrix)

    if "nc" not in _NC_CACHE:
        _NC_CACHE["nc"] = _build_nc()
    nc = _NC_CACHE["nc"]

    shards = np.split(u0_bf, N_CORES, axis=0)
    in_maps = [{"u": s, "bd": bd} for s in shards]
    res = run_bass_kernel_spmd(
        nc, in_maps, core_ids=list(range(N_CORES)), trace=TRACE
    )
    LAST_RESULTS = res

    outs = []
    for r in res.results:
        arr = np.asarray(r["out"])  # [128, 32768] bf16, k=(r,f) interleaved
        y = (
            arr.reshape(2, NX, PAIRS)
            .transpose(2, 0, 1)
            .reshape(ROWS_PER_CORE, NX)
            .astype(np.float32)
        )
        outs.append(y)
    return np.concatenate(outs, axis=0)


# revision 13
# speedup vs baseline: 1.9125x; 1.0444x over previous
"""Trainium2 Bass kernel for nn_HeatEquation1D.

The reference applies a fixed 62x62 Crank-Nicolson step matrix 100 times to
u0[:, 1:-1] via lax.scan, then zero-pads the boundary columns.  Algebraically
that whole scan is a single matmul:

    out = u0 @ W64,   W64[1:63, 1:63] = (step_matrix^100).T,  zero elsewhere

W64 is computed on the host in float64.  The rel-err budget (2e-2) admits
bf16: the host rounds u0 to bf16 (halving the device's read traffic), the
matmul runs bf16 x bf16 -> f32, and the device stores bf16 which the host
upcasts to f32 (halving write traffic).  Measured rel err ~2.8e-3.

Device kernel (per core, pure data parallel over 8 cores):
  - u (bf16, 65536 x 64) is viewed as [32768 row-pairs, 128] and loaded with
    HW DMA-transpose (sync ring) into SBUF tiles T = [128, 1024]: partition
    k = (r, f) interleaved (r = row-within-pair, f = feature), free = row-pair.
    This IS the transposed-chunk layout the matmul needs -- no PE transposes,
    no PSUM round-trip for the transpose.
  - One constant stationary BD = block_diag(W64, W64) (bf16, symmetric);
    matmul(yp, lhsT=BD, rhs=T[:, j:j+512]) gives Y in the same interleaved
    layout: yp[(r, f'), n] = (u @ W64)[2n+r, f'].
  - DVE and ACT alternately copy yp (PSUM f32) -> ys (SBUF bf16).
  - Stores (scalar ring, overlapping the loads' sync ring) write ys to a
    bf16 DRAM tensor out[128, 32768]; the host un-interleaves
    out[(r, f'), p] -> y[2p+r, f'] and upcasts to f32.

Per-core HBM traffic: 2 x 8.39 MB; the two DMA streams overlap on separate
HWDGE rings.  32 blocks of 256 KiB with deep tile pools keep the pipeline
throughput-bound instead of latency-bound.
"""

import numpy as np
import ml_dtypes

BATCH = 524288
NX = 64
NUM_STEPS = 100
N_CORES = 8
ROWS_PER_CORE = BATCH // N_CORES          # 65536
P = 128
PAIRS = ROWS_PER_CORE // 2                 # 32768 row-pairs per core

BLOCK_PAIRS = 4096                         # row-pairs per load/store DMA (1 MiB bf16)
N_BLOCKS = PAIRS // BLOCK_PAIRS            # 8 loads + 8 stores + 1 const = 17 DMAs
MM_N = 512                                 # matmul moving free dim (fp32 out -> 1 bank)

TRACE = False
LAST_RESULTS = None

_NC_CACHE = {}


def _build_nc():
    from concourse import bacc, mybir
    from concourse.tile import TileContext

    nc = bacc.Bacc("TRN2", target_bir_lowering=False, debug=False)
    f32 = mybir.dt.float32
    bf16 = mybir.dt.bfloat16

    u = nc.dram_tensor("u", [ROWS_PER_CORE, NX], bf16, kind="ExternalInput")
    bd_d = nc.dram_tensor("bd", [P, P], bf16, kind="ExternalInput")
    out = nc.dram_tensor("out", [P, PAIRS], bf16, kind="ExternalOutput")

    u_r = u.rearrange("(nb rp r) f -> nb rp (r f)", rp=BLOCK_PAIRS, r=2)
    out_r = out.rearrange("k (nb rp) -> nb k rp", rp=BLOCK_PAIRS)

    with TileContext(nc) as tc:
        with (
            tc.tile_pool(name="consts", bufs=1) as cpool,
            tc.tile_pool(name="xin", bufs=5) as xpool,
            tc.tile_pool(name="yout", bufs=5) as ypool,
            tc.tile_pool(name="ps_y", bufs=4, space="PSUM") as psy,
        ):
            bd_s = cpool.tile([P, P], bf16)
            nc.gpsimd.dma_start(out=bd_s[:], in_=bd_d[:])

            for nb in range(N_BLOCKS):
                x = xpool.tile([P, BLOCK_PAIRS], bf16)
                nc.sync.dma_start(out=x[:], in_=u_r[nb], transpose=True)

                ys = ypool.tile([P, BLOCK_PAIRS], bf16)
                for j in range(BLOCK_PAIRS // (2 * MM_N)):
                    yp = psy.tile([P, 2, MM_N], f32)
                    for m in range(2):
                        c0 = (2 * j + m) * MM_N
                        nc.tensor.matmul(
                            yp[:, m],
                            bd_s[:],
                            x[:, c0 : c0 + MM_N],
                            start=True,
                            stop=True,
                        )
                    dst = ys[:, 2 * j * MM_N : 2 * (j + 1) * MM_N]
                    if j % 2 == 0:
                        nc.vector.tensor_copy(out=dst, in_=yp[:])
                    else:
                        nc.scalar.copy(out=dst, in_=yp[:])
                nc.gpsimd.dma_start(out=out_r[nb], in_=ys[:])

    nc.compile()
    return nc


def _host_matrix(step_matrix):
    m = np.asarray(step_matrix, dtype=np.float64)
    w_inner = np.linalg.matrix_power(m, NUM_STEPS).T  # right-multiplier, f64
    w64 = np.zeros((NX, NX), dtype=np.float64)
    w64[1 : NX - 1, 1 : NX - 1] = w_inner
    bd = np.zeros((P, P), dtype=np.float64)
    bd[:NX, :NX] = w64
    bd[NX:, NX:] = w64
    return bd.astype(ml_dtypes.bfloat16)


def kernel(u0, step_matrix):
    global LAST_RESULTS
    from concourse.bass_utils import run_bass_kernel_spmd

    u0 = np.asarray(u0)
    assert u0.shape == (BATCH, NX), u0.shape
    u0_bf = np.ascontiguousarray(u0.astype(ml_dtypes.bfloat16))

    bd = _host_matrix(step_matrix)

    if "nc" not in _NC_CACHE:
        _NC_CACHE["nc"] = _build_nc()
    nc = _NC_CACHE["nc"]

    shards = np.split(u0_bf, N_CORES, axis=0)
    in_maps = [{"u": s, "bd": bd} for s in shards]
    res = run_bass_kernel_spmd(
        nc, in_maps, core_ids=list(range(N_CORES)), trace=TRACE
    )
    LAST_RESULTS = res

    outs = []
    for r in res.results:
        arr = np.asarray(r["out"])  # [128, 32768] bf16, k=(r,f) interleaved
        y = (
            arr.reshape(2, NX, PAIRS)
            .transpose(2, 0, 1)
            .reshape(ROWS_PER_CORE, NX)
            .astype(np.float32)
        )
        outs.append(y)
    return np.concatenate(outs, axis=0)
